# revision 1
# baseline (speedup 1.0000x reference)
"""Trainium2 Bass kernel for neighbor-sum aggregation (GNN message passing).

reference:  out[b, :] = sum_k embed_matrix[neigh_idx[b, k], :]   (B=50000, K=16,
            U=100000, D=512)

Strategy: data-parallel over B across 8 NeuronCores (embed_matrix replicated).
Each core processes 6272 rows (B padded 50000 -> 50176 with dummy index-0 rows)
as 49 tiles of 128 rows. Per tile, ONE SWDGE indirect DMA gathers all 128x16
neighbor rows (2048 descriptors, 4 MB) into an SBUF tile laid out
[128 partitions, 16*512 f32]; a 4-level in-place tree of DVE adds reduces the
16 chunks to the 512-wide output row, which is stored with a HWDGE DMA.
"""

import numpy as np

import concourse.bacc as bacc
import concourse.bass as bass
import concourse.mybir as mybir
import concourse.tile as tile
from concourse.bass_utils import run_bass_kernel_spmd

N_CORES = 8
B, K = 50000, 16
U, D = 100000, 512
P = 128
TILES = 49                      # output tiles per core
B_SHARD = TILES * P             # 6272 padded rows per core
B_PAD = N_CORES * B_SHARD       # 50176

GATH_BUFS = 3
ACC_BUFS = 3

_NC_CACHE = {}


def build_nc(reps=1):
    """reps>1 wraps the whole tile loop in a hardware For_i for benchmarking."""
    nc = bacc.Bacc("TRN2", target_bir_lowering=False, debug=False)
    idx = nc.dram_tensor("idx", [B_SHARD, K], mybir.dt.int32, kind="ExternalInput")
    embed = nc.dram_tensor("embed", [U, D], mybir.dt.float32, kind="ExternalInput")
    out = nc.dram_tensor("out", [B_SHARD, D], mybir.dt.float32, kind="ExternalOutput")

    with tile.TileContext(nc) as tc:
        with (
            tc.tile_pool(name="idxp", bufs=1) as idx_pool,
            tc.tile_pool(name="gath", bufs=GATH_BUFS) as gpool,
            tc.tile_pool(name="accp", bufs=ACC_BUFS) as apool,
        ):
            # All indices up-front in one DMA: idx_all[p, t*K+k] = idx[t*128+p, k]
            idx_all = idx_pool.tile([P, TILES * K], mybir.dt.int32)
            nc.sync.dma_start(
                out=idx_all[:].rearrange("p (t k) -> p t k", k=K),
                in_=idx.ap().rearrange("(t p) k -> p t k", p=P),
            )

            def body():
                for t in range(TILES):
                    gath = gpool.tile([P, K * D], mybir.dt.float32, tag="g")
                    # HW indirect DMA: exactly one index per partition per op,
                    # each gathering one contiguous D-row of embed.
                    for k in range(K):
                        nc.gpsimd.indirect_dma_start(
                            out=gath[:, k * D : (k + 1) * D],
                            out_offset=None,
                            in_=embed.ap(),
                            in_offset=bass.IndirectOffsetOnAxis(
                                ap=idx_all[:, t * K + k : t * K + k + 1], axis=0
                            ),
                        )
                    # Single-port DVE reduce over the strided [p][d][k] view —
                    # avoids 2-port tensor_tensor ops that contend with the
                    # GpSimd SWDGE descriptor path for the shared SBUF port.
                    acc = apool.tile([P, D], mybir.dt.float32, tag="a")
                    nc.vector.tensor_reduce(
                        out=acc[:],
                        in_=gath[:].rearrange("p (k d) -> p d k", d=D),
                        axis=mybir.AxisListType.X,
                        op=mybir.AluOpType.add,
                    )
                    nc.sync.dma_start(
                        out=out.ap()[t * P : (t + 1) * P, :], in_=acc[:]
                    )

            if reps == 1:
                body()
            else:
                with tc.For_i(0, reps, 1):
                    body()
    nc.compile()
    return nc


def _get_nc():
    if "nc" not in _NC_CACHE:
        _NC_CACHE["nc"] = build_nc()
    return _NC_CACHE["nc"]


def _run(nc, in_maps, **kwargs):
    return run_bass_kernel_spmd(nc, in_maps, list(range(N_CORES)), **kwargs)


def make_in_maps(neigh_idx, embed_matrix):
    idx = np.asarray(neigh_idx).astype(np.int32)
    embed = np.ascontiguousarray(np.asarray(embed_matrix), dtype=np.float32)
    idx_pad = np.zeros((B_PAD, K), np.int32)
    idx_pad[:B] = idx
    shards = idx_pad.reshape(N_CORES, B_SHARD, K)
    return [
        {"idx": np.ascontiguousarray(shards[c]), "embed": embed}
        for c in range(N_CORES)
    ]


def kernel(neigh_idx, embed_matrix):
    nc = _get_nc()
    in_maps = make_in_maps(neigh_idx, embed_matrix)
    res = _run(nc, in_maps).results
    out = np.concatenate([res[c]["out"] for c in range(N_CORES)], axis=0)[:B]
    return np.ascontiguousarray(out, dtype=np.float32)

